# revision 16
# baseline (speedup 1.0000x reference)
"""Trainium2 Bass kernel for nn_DynamicConv (dense_cnn).

out[i, j, co, h, w] = sum_k (conv_k(x_i)[co, h, w] + b_k[co]) * attn[j, k]
attn = softmax(softmax(MLP(meanpool(x)), k) / TAU, k)

Sharding: data-parallel over batch i across 8 cores, with NO cross-core
collective.  The attention matrix needs pooled vectors of ALL samples, so
every core receives the full batch in bf16 (4.6 MB) and computes the whole
[B, K] attention locally.  A runtime AllGather was measured to cost
15-105us per core purely in launch-skew rendezvous; replicating the input
removes that entirely and makes the cores embarrassingly parallel.

Per-core inputs are ROTATED so slot 0 is the core's own sample (shipped
pre-padded for the conv); the host un-rotates the output slabs (np.roll)
when gathering.

Everything on the PE runs in bf16 (fp32 PSUM accumulate): conv as 9
shifted matmuls over the padded image, then the cross-batch blend as a
block-diagonal matmul per 16-channel group.  The block-diagonal blend
matrix BD is built on-chip as (P4.T @ broadcast(attn.T)) * M01 with two
tiny constants, avoiding 32 scatter DMAs.

Scheduling notes (from perfetto traces):
- all loads go on the sync HWDGE queue; scalar carries zero DMAs (a DMA
  backlog there once blocked conv PSUM evictions for 15us);
- all output stores go on the gpsimd software queue, which fans out over
  all 16 DMA engines (the HWDGE queues only reach 8 and cap at
  ~190 GB/s);
- blend u-halves are spliced between conv row-groups one at a time: the
  in-order PE queue means a blend-only stretch runs at PSUM-drain pace
  (~3.2 chunks/us vs 4.2 produced) and drops the PE out of its 2.4 GHz
  pstate, so bursts are kept to <=2 u-halves (psB absorbs the backlog);
- the first blend waits for attn (~34us: 8 serial 2.5us DVE reduces +
  MLP), so blends start only at conv2-end (~41us).
"""

import sys

import numpy as np

if "/opt/trn_rl_repo" not in sys.path:
    sys.path.insert(0, "/opt/trn_rl_repo")

import concourse.bacc as bacc
import concourse.bass as bass
import concourse.mybir as mybir
import concourse.tile as tile

F32 = mybir.dt.float32
BF = mybir.dt.bfloat16
AF = mybir.ActivationFunctionType
AX = mybir.AxisListType
ALU = mybir.AluOpType

B = 8
CIN = 128
COUT = 256
K = 4
KS = 3
HW = 48
HW2 = HW * HW          # 2304
WP = HW + 2            # 50 (padded)
WP2 = WP * WP          # 2500
HID = 256
TAU = 30.0
NCORES = 8

ROW_GROUPS = [(0, 10), (10, 10), (20, 10), (30, 10), (40, 8)]
CHUNKS = [(0, 512), (512, 512), (1024, 512), (1536, 512), (2048, 256)]


def build_nc():
    nc = bacc.Bacc("TRN2", debug=False, num_devices=NCORES)

    # own sample, zero-padded to [128, 50*50] on the host
    xp0d = nc.dram_tensor("xp0", [CIN, WP2], BF, kind="ExternalInput").ap()
    # slots 1..7 hold samples (core + j) % 8
    xrest = nc.dram_tensor("xrest", [7 * CIN, HW2], BF, kind="ExternalInput").ap()
    # [ci, t, tap, p] flattened; p = c*4 + k encodes (co = 32 t + c, k)
    wconv = nc.dram_tensor("wconv", [CIN, 8 * 9 * 128], BF, kind="ExternalInput").ap()
    bconv = nc.dram_tensor("bconv", [128, 8], F32, kind="ExternalInput").ap()
    w1t = nc.dram_tensor("w1t", [CIN, HID], BF, kind="ExternalInput").ap()
    b1c = nc.dram_tensor("b1c", [128, 2], F32, kind="ExternalInput").ap()
    w2t = nc.dram_tensor("w2t", [128, 2 * K], BF, kind="ExternalInput").ap()
    b2r = nc.dram_tensor("b2r", [1, K], BF, kind="ExternalInput").ap()
    one18 = nc.dram_tensor("one18", [1, B], BF, kind="ExternalInput").ap()
    ident8 = nc.dram_tensor("ident8", [B, B], BF, kind="ExternalInput").ap()
    # p4[k, 64u + 4c + k] = 1: scatters attn.T rows onto the (c, k) comb
    p4d = nc.dram_tensor("p4", [K, 128], BF, kind="ExternalInput").ap()
    # m01[64u + 4c + k, 128u' + 16j + c'] = (u == u') & (c == c')
    m01d = nc.dram_tensor("m01", [128, 256], BF, kind="ExternalInput").ap()
    out = nc.dram_tensor("out", [B, COUT, HW2], F32, kind="ExternalOutput").ap()

    with tile.TileContext(nc, num_cores=NCORES) as tc:
        with (
            tc.tile_pool(name="const", bufs=1) as const,
            tc.tile_pool(name="csb", bufs=8) as csb_pool,
            tc.tile_pool(name="osb", bufs=5) as osb_pool,
            tc.tile_pool(name="psA", bufs=2, space="PSUM") as psA,
            tc.tile_pool(name="psB", bufs=5, space="PSUM") as psB,
            tc.tile_pool(name="psM", bufs=1, space="PSUM") as psM,
        ):
            # ---- loads: ALL on the sync HWDGE queue, ordered by first use
            xp = const.tile([128, WP2], BF)
            xall_sb = const.tile([128, 7 * HW2], BF)
            wt = []
            for t in range(8):
                w = const.tile([128, 9 * 128], BF, tag=f"wt{t}")
                wt.append(w)
            bct = const.tile([128, 8], F32)
            w1s = const.tile([128, HID], BF)
            b1s = const.tile([128, 2], F32)
            w2s = const.tile([128, 2 * K], BF)
            b2s = const.tile([1, K], BF)
            ones = const.tile([1, B], BF)
            id8 = const.tile([B, B], BF)
            p4s = const.tile([K, 128], BF)
            m01 = const.tile([128, 256], BF)

            def ldx(j):
                nc.sync.dma_start(
                    xall_sb[:, (j - 1) * HW2 : j * HW2],
                    xrest[(j - 1) * 128 : j * 128, :],
                )

            def ldw(t):
                nc.sync.dma_start(
                    wt[t][:], wconv[:, t * 9 * 128 : (t + 1) * 9 * 128]
                )

            nc.sync.dma_start(xp[:], xp0d[:, :])
            ldw(0)
            nc.sync.dma_start(bct[:], bconv[:, :])
            ldw(1)
            for j in range(1, 8):
                ldx(j)
            nc.sync.dma_start(w1s[:], w1t[:, :])
            nc.sync.dma_start(w2s[:], w2t[:, :])
            ldw(2)
            nc.sync.dma_start(b1s[:], b1c[:, :])
            nc.sync.dma_start(b2s[:], b2r[:, :])
            nc.sync.dma_start(ones[:], one18[:, :])
            nc.sync.dma_start(id8[:], ident8[:, :])
            nc.sync.dma_start(p4s[:], p4d[:, :])
            nc.sync.dma_start(m01[:], m01d[:, :])
            for t in range(3, 8):
                ldw(t)

            # pre-warm the ACT function tables (1.3us each if loaded lazily
            # inside the latency-critical chains); reads the padded zeros
            actw = const.tile([128, 1], F32)
            zcol = xp[:, 0:2].bitcast(F32)[:, 0:1]
            nc.scalar.activation(actw[:], zcol, AF.Identity, bias=zcol)
            nc.scalar.copy(actw[:], zcol)
            nc.scalar.activation(actw[:], zcol, AF.Relu, bias=zcol)
            nc.scalar.activation(actw[:], zcol, AF.Exp, bias=zcol)

            xp3 = xp[:].rearrange("p (h w) -> p h w", w=WP)

            # ---- global average pooling of all 8 samples (1/HW2 in w1t).
            # slot 0 sums the padded image (same sum).  2.5us per reduce,
            # serial on DVE -- blends only start at conv2-end (~41us)
            pooled8 = const.tile([128, B], BF)
            with nc.allow_low_precision(reason="bf16 matmul operand"):
                nc.vector.tensor_reduce(
                    pooled8[:, 0:1], xp[:], axis=AX.X, op=ALU.add
                )
                for j in range(1, 8):
                    nc.vector.tensor_reduce(
                        pooled8[:, j : j + 1],
                        xall_sb[:, (j - 1) * HW2 : j * HW2],
                        axis=AX.X,
                        op=ALU.add,
                    )

            cs_tiles = [None] * 8

            def emit_conv(t, groups):
                if cs_tiles[t] is None:
                    cs = csb_pool.tile([128, HW2], BF, tag="csb")
                    cs_tiles[t] = cs
                cs = cs_tiles[t]
                for gi in groups:
                    r0, R = ROW_GROUPS[gi]
                    pt = psA.tile([128, R * HW], F32, tag="cps")
                    for tap in range(9):
                        dh, dw = divmod(tap, 3)
                        rhs = xp3[:, r0 + dh : r0 + dh + R, dw : dw + HW]
                        nc.tensor.matmul(
                            pt[:],
                            lhsT=wt[t][:, tap * 128 : (tap + 1) * 128],
                            rhs=rhs,
                            start=(tap == 0),
                            stop=(tap == 8),
                        )
                    # PSUM -> SBUF eviction, fused with the conv bias add
                    nc.scalar.activation(
                        cs[:, r0 * HW : (r0 + R) * HW],
                        pt[:],
                        AF.Identity,
                        bias=bct[:, t : t + 1],
                    )

            def emit_blend_u(t, u, BD):
                cs = cs_tiles[t]
                g = 2 * t + u
                ob = osb_pool.tile([128, HW2], F32, tag="osb")
                for ci_, (c0, C) in enumerate(CHUNKS):
                    bp = psB.tile([128, C], F32, tag="bps")
                    nc.tensor.matmul(
                        bp[:],
                        lhsT=BD[:, 128 * u : 128 * u + 128],
                        rhs=cs[:, c0 : c0 + C],
                        start=True,
                        stop=True,
                    )
                    # PSUM drain balanced across DVE and ACT so psB bank
                    # recycling (not one engine) sets the blend rate
                    if ci_ in (1, 4):
                        nc.scalar.copy(ob[:, c0 : c0 + C], bp[:])
                    else:
                        nc.vector.tensor_copy(ob[:, c0 : c0 + C], bp[:])
                dst = out[:, 16 * g : 16 * g + 16, :]
                if g >= 14:
                    # tail stores are the critical path: split across all
                    # three queues (~1.2us instead of 3.3us each)
                    d3 = dst.rearrange("b c (s x) -> b c s x", s=3)
                    o3 = ob[:].rearrange("p (s x) -> p s x", s=3)
                    nc.gpsimd.dma_start(d3[:, :, 0, :], o3[:, 0, :])
                    nc.sync.dma_start(d3[:, :, 1, :], o3[:, 1, :])
                    nc.scalar.dma_start(d3[:, :, 2, :], o3[:, 2, :])
                elif g % 3 == 2:
                    # the sync HWDGE queue is idle once loads finish (~25us);
                    # putting 1/3 of stores there keeps the gpsimd queue's
                    # serial transfer time (3.2us per store) off the tail
                    nc.sync.dma_start(dst, ob[:])
                else:
                    nc.gpsimd.dma_start(dst, ob[:])

            FIRST = [0, 1, 2]
            SECOND = [3, 4]

            emit_conv(0, FIRST)
            emit_conv(0, SECOND)
            emit_conv(1, FIRST)
            emit_conv(1, SECOND)

            # ---- attention MLP + double softmax for all 8 samples ----
            hd = []
            for h in range(2):
                hps = psM.tile([128, B], F32, tag="mlp")
                nc.tensor.matmul(
                    hps[:],
                    lhsT=w1s[:, h * 128 : (h + 1) * 128],
                    rhs=pooled8[:],
                    start=True,
                    stop=True,
                )
                hsb = const.tile([128, B], BF, tag=f"hd{h}")
                nc.scalar.activation(hsb[:], hps[:], AF.Relu, bias=b1s[:, h : h + 1])
                hd.append(hsb)

            lps = psM.tile([B, K], F32, tag="mlp")
            nc.tensor.matmul(
                lps[:], lhsT=hd[0][:], rhs=w2s[:, 0:K], start=True, stop=False
            )
            nc.tensor.matmul(
                lps[:], lhsT=hd[1][:], rhs=w2s[:, K : 2 * K], start=False, stop=False
            )
            nc.tensor.matmul(
                lps[:], lhsT=ones[:], rhs=b2s[:], start=False, stop=True
            )

            # double softmax over k (shift-invariant: max-subtraction dropped)
            e1 = const.tile([B, K], F32)
            nc.scalar.activation(e1[:], lps[:], AF.Exp, bias=0.0, scale=1.0)
            s1 = const.tile([B, 1], F32)
            nc.vector.tensor_reduce(s1[:], e1[:], axis=AX.X, op=ALU.add)
            r1 = const.tile([B, 1], F32)
            nc.vector.reciprocal(r1[:], s1[:])
            a1 = const.tile([B, K], F32)
            nc.vector.tensor_scalar_mul(a1[:], e1[:], r1[:, 0:1])

            e2 = const.tile([B, K], F32)
            nc.scalar.activation(e2[:], a1[:], AF.Exp, bias=0.0, scale=1.0 / TAU)
            s2 = const.tile([B, 1], F32)
            nc.vector.tensor_reduce(s2[:], e2[:], axis=AX.X, op=ALU.add)
            r2 = const.tile([B, 1], F32)
            nc.vector.reciprocal(r2[:], s2[:])
            attn_bf = const.tile([B, K], BF)
            with nc.allow_low_precision(reason="bf16 blend operand"):
                nc.vector.tensor_scalar_mul(attn_bf[:], e2[:], r2[:, 0:1])

            # attn [j, k] -> attn_T [k, j] via PE transpose
            tps = psM.tile([K, B], BF, tag="mlp")
            nc.tensor.transpose(tps[:], attn_bf[:], id8[:])
            atT = const.tile([K, B], BF)
            nc.scalar.copy(atT[:], tps[:])

            # BD[64u+4c+k, 128u'+16j+c'] = attn[j, k] * (u==u') * (c==c'):
            # comb matmul broadcasts attn.T to every (c, u) slot, the mask
            # kills the off-diagonal (c != c') copies
            atRep = const.tile([K, 256], BF)
            at4 = atT[:].rearrange("k (u j c) -> k u j c", u=1, c=1)
            nc.vector.tensor_copy(
                atRep[:].rearrange("k (u j c) -> k u j c", u=2, c=16),
                at4.broadcast_to([K, 2, 8, 16]),
            )
            psD = psM.tile([128, 256], F32, tag="mlp")
            nc.tensor.matmul(psD[:], lhsT=p4s[:], rhs=atRep[:], start=True, stop=True)
            BD = const.tile([128, 256], BF)
            with nc.allow_low_precision(reason="bf16 blend operand"):
                nc.vector.tensor_tensor(BD[:], psD[:], m01[:], op=ALU.mult)

            # splice blend u-halves between conv half-slots, at most two
            # consecutively, so PSUM drains always overlap conv matmuls
            emit_conv(2, FIRST)
            emit_conv(2, SECOND)
            emit_blend_u(0, 0, BD)
            emit_blend_u(0, 1, BD)
            emit_conv(3, FIRST)
            emit_blend_u(1, 0, BD)
            emit_conv(3, SECOND)
            emit_blend_u(1, 1, BD)
            emit_conv(4, FIRST)
            emit_blend_u(2, 0, BD)
            emit_conv(4, SECOND)
            emit_blend_u(2, 1, BD)
            emit_conv(5, FIRST)
            emit_blend_u(3, 0, BD)
            emit_conv(5, SECOND)
            emit_blend_u(3, 1, BD)
            emit_blend_u(4, 0, BD)
            emit_conv(6, FIRST)
            emit_blend_u(4, 1, BD)
            emit_conv(6, SECOND)
            emit_blend_u(5, 0, BD)
            emit_blend_u(5, 1, BD)
            emit_conv(7, FIRST)
            emit_blend_u(6, 0, BD)
            emit_conv(7, SECOND)
            emit_blend_u(6, 1, BD)
            emit_blend_u(7, 0, BD)
            emit_blend_u(7, 1, BD)

    nc.compile()
    return nc


def pack_inputs(x, conv_w, conv_b, w1, b1, w2, b2):
    """Host-side layout packing (dtype casts, zero-padding, constant folds)."""
    import ml_dtypes

    bf16 = ml_dtypes.bfloat16
    x_bf = np.asarray(x, dtype=np.float32).reshape(B, CIN, HW, HW).astype(bf16)
    xpad = np.zeros((B, CIN, WP, WP), dtype=bf16)
    xpad[:, :, 1 : 1 + HW, 1 : 1 + HW] = x_bf

    # conv_w [K, COUT, CIN, 3, 3] -> [ci, t, tap, p] with p = c*4 + k,
    # co = 32 t + c
    w = np.asarray(conv_w, dtype=np.float32).transpose(2, 3, 4, 0, 1)  # ci kh kw k co
    w = w.reshape(CIN, KS, KS, K, 8, 32)  # ci kh kw k t c
    w = w.transpose(0, 4, 1, 2, 5, 3)  # ci t kh kw c k
    wconv = np.ascontiguousarray(w.reshape(CIN, 8 * 9 * 128)).astype(bf16)

    bc = np.asarray(conv_b, dtype=np.float32).reshape(K, 8, 32)  # k t c
    bconv = np.ascontiguousarray(bc.transpose(1, 2, 0).reshape(8, 128).T)  # [p, t]

    w1t = (np.ascontiguousarray(np.asarray(w1, dtype=np.float32).T) / float(HW2)).astype(bf16)
    b1c = np.ascontiguousarray(np.asarray(b1, dtype=np.float32).reshape(2, 128).T)
    w2T = np.asarray(w2, dtype=np.float32).T  # [256, 4]
    w2t = np.ascontiguousarray(np.concatenate([w2T[:128], w2T[128:]], axis=1)).astype(bf16)
    b2r = np.asarray(b2, dtype=np.float32).reshape(1, K).astype(bf16)

    p4 = np.zeros((K, 128), dtype=np.float32)
    m01 = np.zeros((128, 256), dtype=np.float32)
    for u in range(2):
        for c in range(16):
            for k in range(K):
                p4[k, 64 * u + 4 * c + k] = 1.0
                m01[64 * u + 4 * c + k, 128 * u + 16 * np.arange(8) + c] = 1.0

    common = dict(
        wconv=wconv, bconv=bconv, w1t=w1t, b1c=b1c, w2t=w2t, b2r=b2r,
        one18=np.ones((1, B), dtype=np.float32).astype(bf16),
        ident8=np.eye(B, dtype=np.float32).astype(bf16),
        p4=p4.astype(bf16), m01=m01.astype(bf16),
    )
    in_maps = [
        dict(
            common,
            xp0=np.ascontiguousarray(xpad[i].reshape(CIN, WP2)),
            xrest=np.ascontiguousarray(
                np.roll(x_bf, -i, axis=0)[1:].reshape(7 * CIN, HW2)
            ),
        )
        for i in range(NCORES)
    ]
    return in_maps


def run(inputs, trace=False):
    from concourse.bass_utils import run_bass_kernel_spmd

    nc = build_nc()
    in_maps = pack_inputs(**inputs)
    res = run_bass_kernel_spmd(
        nc, in_maps, core_ids=list(range(NCORES)), trace=trace
    )
    # core i's slab row q holds sample j = (i + q) % 8: un-rotate
    slabs = [np.roll(res.results[i]["out"], i, axis=0) for i in range(NCORES)]
    out = np.stack(slabs, axis=0).reshape(B, B, COUT, HW, HW)
    return out, res


def kernel(**inputs) -> np.ndarray:
    out, _ = run(inputs, trace=False)
    return out


# revision 18
# speedup vs baseline: 1.1505x; 1.1505x over previous
"""Trainium2 Bass kernel for nn_DynamicConv (dense_cnn).

out[i, j, co, h, w] = sum_k (conv_k(x_i)[co, h, w] + b_k[co]) * attn[j, k]
attn = softmax(softmax(MLP(meanpool(x)), k) / TAU, k)

Sharding: data-parallel over batch i across 8 cores, with NO cross-core
collective.  The attention matrix needs pooled vectors of ALL samples, so
every core receives the full batch in bf16 (4.6 MB) and computes the whole
[B, K] attention locally.  A runtime AllGather was measured to cost
15-105us per core purely in launch-skew rendezvous; replicating the input
removes that entirely and makes the cores embarrassingly parallel.

Per-core inputs are ROTATED so slot 0 is the core's own sample (shipped
pre-padded for the conv); the host un-rotates the output slabs (np.roll)
when gathering.

Everything on the PE runs in bf16 (fp32 PSUM accumulate): conv as 9
shifted matmuls over the padded image, then the cross-batch blend as a
block-diagonal matmul per 16-channel group.  The block-diagonal blend
matrix BD is built on-chip as (P4.T @ broadcast(attn.T)) * M01 with two
tiny constants, avoiding 32 scatter DMAs.

Scheduling notes (from perfetto traces):
- all loads go on the sync HWDGE queue; scalar carries zero DMAs (a DMA
  backlog there once blocked conv PSUM evictions for 15us);
- all output stores go on the gpsimd software queue, which fans out over
  all 16 DMA engines (the HWDGE queues only reach 8 and cap at
  ~190 GB/s);
- blend u-halves are spliced between conv row-groups one at a time: the
  in-order PE queue means a blend-only stretch runs at PSUM-drain pace
  (~3.2 chunks/us vs 4.2 produced) and drops the PE out of its 2.4 GHz
  pstate, so bursts are kept to <=2 u-halves (psB absorbs the backlog);
- the first blend waits for attn (~34us: 8 serial 2.5us DVE reduces +
  MLP), so blends start only at conv2-end (~41us).
"""

import sys

import numpy as np

if "/opt/trn_rl_repo" not in sys.path:
    sys.path.insert(0, "/opt/trn_rl_repo")

import concourse.bacc as bacc
import concourse.bass as bass
import concourse.mybir as mybir
import concourse.tile as tile

F32 = mybir.dt.float32
BF = mybir.dt.bfloat16
AF = mybir.ActivationFunctionType
AX = mybir.AxisListType
ALU = mybir.AluOpType

B = 8
CIN = 128
COUT = 256
K = 4
KS = 3
HW = 48
HW2 = HW * HW          # 2304
WP = HW + 2            # 50 (padded)
WP2 = WP * WP          # 2500
HID = 256
TAU = 30.0
NCORES = 8

ROW_GROUPS = [(0, 10), (10, 10), (20, 10), (30, 10), (40, 8)]
CHUNKS = [(0, 512), (512, 512), (1024, 512), (1536, 512), (2048, 256)]


def build_nc():
    nc = bacc.Bacc("TRN2", debug=False, num_devices=NCORES)

    # own sample, zero-padded to [128, 50*50] on the host
    xp0d = nc.dram_tensor("xp0", [CIN, WP2], BF, kind="ExternalInput").ap()
    # slots 1..7 hold samples (core + j) % 8
    xrest = nc.dram_tensor("xrest", [7 * CIN, HW2], BF, kind="ExternalInput").ap()
    # [ci, t, tap, p] flattened; p = c*4 + k encodes (co = 32 t + c, k)
    wconv = nc.dram_tensor("wconv", [CIN, 8 * 9 * 128], BF, kind="ExternalInput").ap()
    bconv = nc.dram_tensor("bconv", [128, 8], F32, kind="ExternalInput").ap()
    w1t = nc.dram_tensor("w1t", [CIN, HID], BF, kind="ExternalInput").ap()
    b1c = nc.dram_tensor("b1c", [128, 2], F32, kind="ExternalInput").ap()
    w2t = nc.dram_tensor("w2t", [128, 2 * K], BF, kind="ExternalInput").ap()
    b2r = nc.dram_tensor("b2r", [1, K], BF, kind="ExternalInput").ap()
    one18 = nc.dram_tensor("one18", [1, B], BF, kind="ExternalInput").ap()
    ident8 = nc.dram_tensor("ident8", [B, B], BF, kind="ExternalInput").ap()
    # p4[k, 64u + 4c + k] = 1: scatters attn.T rows onto the (c, k) comb
    p4d = nc.dram_tensor("p4", [K, 128], BF, kind="ExternalInput").ap()
    # m01[64u + 4c + k, 128u' + 16j + c'] = (u == u') & (c == c')
    m01d = nc.dram_tensor("m01", [128, 256], BF, kind="ExternalInput").ap()
    out = nc.dram_tensor("out", [B, COUT, HW2], F32, kind="ExternalOutput").ap()

    with tile.TileContext(nc, num_cores=NCORES) as tc:
        with (
            tc.tile_pool(name="const", bufs=1) as const,
            tc.tile_pool(name="csb", bufs=8) as csb_pool,
            tc.tile_pool(name="osb", bufs=8) as osb_pool,
            tc.tile_pool(name="psA", bufs=2, space="PSUM") as psA,
            tc.tile_pool(name="psB", bufs=5, space="PSUM") as psB,
            tc.tile_pool(name="psM", bufs=1, space="PSUM") as psM,
        ):
            # ---- loads: ALL on the sync HWDGE queue, ordered by first use
            xp = const.tile([128, WP2], BF)
            xall_sb = const.tile([128, 7 * HW2], BF)
            wt = []
            for t in range(8):
                w = const.tile([128, 9 * 128], BF, tag=f"wt{t}")
                wt.append(w)
            bct = const.tile([128, 8], F32)
            w1s = const.tile([128, HID], BF)
            b1s = const.tile([128, 2], F32)
            w2s = const.tile([128, 2 * K], BF)
            b2s = const.tile([1, K], BF)
            ones = const.tile([1, B], BF)
            id8 = const.tile([B, B], BF)
            p4s = const.tile([K, 128], BF)
            m01 = const.tile([128, 256], BF)

            def ldx(j):
                nc.sync.dma_start(
                    xall_sb[:, (j - 1) * HW2 : j * HW2],
                    xrest[(j - 1) * 128 : j * 128, :],
                )

            def ldw(t):
                nc.sync.dma_start(
                    wt[t][:], wconv[:, t * 9 * 128 : (t + 1) * 9 * 128]
                )

            nc.sync.dma_start(xp[:], xp0d[:, :])
            ldw(0)
            nc.sync.dma_start(bct[:], bconv[:, :])
            ldw(1)
            for j in range(1, 8):
                ldx(j)
            nc.sync.dma_start(w1s[:], w1t[:, :])
            nc.sync.dma_start(w2s[:], w2t[:, :])
            ldw(2)
            nc.sync.dma_start(b1s[:], b1c[:, :])
            nc.sync.dma_start(b2s[:], b2r[:, :])
            nc.sync.dma_start(ones[:], one18[:, :])
            nc.sync.dma_start(id8[:], ident8[:, :])
            nc.sync.dma_start(p4s[:], p4d[:, :])
            nc.sync.dma_start(m01[:], m01d[:, :])
            for t in range(3, 8):
                ldw(t)

            # pre-warm the ACT function tables (1.3us each if loaded lazily
            # inside the latency-critical chains); reads the padded zeros
            actw = const.tile([128, 1], F32)
            zcol = xp[:, 0:2].bitcast(F32)[:, 0:1]
            nc.scalar.activation(actw[:], zcol, AF.Identity, bias=zcol)
            nc.scalar.copy(actw[:], zcol)
            nc.scalar.activation(actw[:], zcol, AF.Relu, bias=zcol)
            nc.scalar.activation(actw[:], zcol, AF.Exp, bias=zcol)

            xp3 = xp[:].rearrange("p (h w) -> p h w", w=WP)

            # ---- global average pooling of all 8 samples (1/HW2 in w1t).
            # slot 0 sums the padded image (same sum).  2.5us per reduce,
            # serial on DVE -- blends only start at conv2-end (~41us)
            pooled8 = const.tile([128, B], BF)
            with nc.allow_low_precision(reason="bf16 matmul operand"):
                nc.vector.tensor_reduce(
                    pooled8[:, 0:1], xp[:], axis=AX.X, op=ALU.add
                )
                for j in range(1, 8):
                    nc.vector.tensor_reduce(
                        pooled8[:, j : j + 1],
                        xall_sb[:, (j - 1) * HW2 : j * HW2],
                        axis=AX.X,
                        op=ALU.add,
                    )

            cs_tiles = [None] * 8

            def emit_conv(t, groups):
                if cs_tiles[t] is None:
                    cs = csb_pool.tile([128, HW2], BF, tag="csb")
                    cs_tiles[t] = cs
                cs = cs_tiles[t]
                for gi in groups:
                    r0, R = ROW_GROUPS[gi]
                    pt = psA.tile([128, R * HW], F32, tag="cps")
                    for tap in range(9):
                        dh, dw = divmod(tap, 3)
                        rhs = xp3[:, r0 + dh : r0 + dh + R, dw : dw + HW]
                        nc.tensor.matmul(
                            pt[:],
                            lhsT=wt[t][:, tap * 128 : (tap + 1) * 128],
                            rhs=rhs,
                            start=(tap == 0),
                            stop=(tap == 8),
                        )
                    # PSUM -> SBUF eviction, fused with the conv bias add
                    nc.scalar.activation(
                        cs[:, r0 * HW : (r0 + R) * HW],
                        pt[:],
                        AF.Identity,
                        bias=bct[:, t : t + 1],
                    )

            def emit_blend_u(t, u, BD):
                cs = cs_tiles[t]
                g = 2 * t + u
                ob = osb_pool.tile([128, HW2], F32, tag="osb")
                for ci_, (c0, C) in enumerate(CHUNKS):
                    bp = psB.tile([128, C], F32, tag="bps")
                    nc.tensor.matmul(
                        bp[:],
                        lhsT=BD[:, 128 * u : 128 * u + 128],
                        rhs=cs[:, c0 : c0 + C],
                        start=True,
                        stop=True,
                    )
                    # PSUM drain balanced across DVE and ACT so psB bank
                    # recycling (not one engine) sets the blend rate
                    if ci_ in (1, 4):
                        nc.scalar.copy(ob[:, c0 : c0 + C], bp[:])
                    else:
                        nc.vector.tensor_copy(ob[:, c0 : c0 + C], bp[:])
                dst = out[:, 16 * g : 16 * g + 16, :]
                if g >= 14:
                    # tail stores are the critical path: split across all
                    # three queues (~1.2us instead of 3.3us each)
                    d3 = dst.rearrange("b c (s x) -> b c s x", s=3)
                    o3 = ob[:].rearrange("p (s x) -> p s x", s=3)
                    nc.gpsimd.dma_start(d3[:, :, 0, :], o3[:, 0, :])
                    nc.sync.dma_start(d3[:, :, 1, :], o3[:, 1, :])
                    nc.scalar.dma_start(d3[:, :, 2, :], o3[:, 2, :])
                elif g % 4 == 3:
                    # the sync HWDGE queue is idle once loads finish (~25us);
                    # a quarter of the stores there keeps the gpsimd queue's
                    # serial transfer time (3.2us per store) off the tail
                    nc.sync.dma_start(dst, ob[:])
                else:
                    nc.gpsimd.dma_start(dst, ob[:])

            FIRST = [0, 1, 2]
            SECOND = [3, 4]

            emit_conv(0, FIRST)
            emit_conv(0, SECOND)
            emit_conv(1, FIRST)
            emit_conv(1, SECOND)

            # ---- attention MLP + double softmax for all 8 samples ----
            hd = []
            for h in range(2):
                hps = psM.tile([128, B], F32, tag="mlp")
                nc.tensor.matmul(
                    hps[:],
                    lhsT=w1s[:, h * 128 : (h + 1) * 128],
                    rhs=pooled8[:],
                    start=True,
                    stop=True,
                )
                hsb = const.tile([128, B], BF, tag=f"hd{h}")
                nc.scalar.activation(hsb[:], hps[:], AF.Relu, bias=b1s[:, h : h + 1])
                hd.append(hsb)

            lps = psM.tile([B, K], F32, tag="mlp")
            nc.tensor.matmul(
                lps[:], lhsT=hd[0][:], rhs=w2s[:, 0:K], start=True, stop=False
            )
            nc.tensor.matmul(
                lps[:], lhsT=hd[1][:], rhs=w2s[:, K : 2 * K], start=False, stop=False
            )
            nc.tensor.matmul(
                lps[:], lhsT=ones[:], rhs=b2s[:], start=False, stop=True
            )

            # double softmax over k (shift-invariant: max-subtraction dropped)
            e1 = const.tile([B, K], F32)
            nc.scalar.activation(e1[:], lps[:], AF.Exp, bias=0.0, scale=1.0)
            s1 = const.tile([B, 1], F32)
            nc.vector.tensor_reduce(s1[:], e1[:], axis=AX.X, op=ALU.add)
            r1 = const.tile([B, 1], F32)
            nc.vector.reciprocal(r1[:], s1[:])
            a1 = const.tile([B, K], F32)
            nc.vector.tensor_scalar_mul(a1[:], e1[:], r1[:, 0:1])

            e2 = const.tile([B, K], F32)
            nc.scalar.activation(e2[:], a1[:], AF.Exp, bias=0.0, scale=1.0 / TAU)
            s2 = const.tile([B, 1], F32)
            nc.vector.tensor_reduce(s2[:], e2[:], axis=AX.X, op=ALU.add)
            r2 = const.tile([B, 1], F32)
            nc.vector.reciprocal(r2[:], s2[:])
            attn_bf = const.tile([B, K], BF)
            with nc.allow_low_precision(reason="bf16 blend operand"):
                nc.vector.tensor_scalar_mul(attn_bf[:], e2[:], r2[:, 0:1])

            # attn [j, k] -> attn_T [k, j] via PE transpose
            tps = psM.tile([K, B], BF, tag="mlp")
            nc.tensor.transpose(tps[:], attn_bf[:], id8[:])
            atT = const.tile([K, B], BF)
            nc.scalar.copy(atT[:], tps[:])

            # BD[64u+4c+k, 128u'+16j+c'] = attn[j, k] * (u==u') * (c==c'):
            # comb matmul broadcasts attn.T to every (c, u) slot, the mask
            # kills the off-diagonal (c != c') copies
            atRep = const.tile([K, 256], BF)
            at4 = atT[:].rearrange("k (u j c) -> k u j c", u=1, c=1)
            nc.vector.tensor_copy(
                atRep[:].rearrange("k (u j c) -> k u j c", u=2, c=16),
                at4.broadcast_to([K, 2, 8, 16]),
            )
            psD = psM.tile([128, 256], F32, tag="mlp")
            nc.tensor.matmul(psD[:], lhsT=p4s[:], rhs=atRep[:], start=True, stop=True)
            BD = const.tile([128, 256], BF)
            with nc.allow_low_precision(reason="bf16 blend operand"):
                nc.vector.tensor_tensor(BD[:], psD[:], m01[:], op=ALU.mult)

            # splice blend u-halves between conv half-slots, at most two
            # consecutively, so PSUM drains always overlap conv matmuls
            emit_conv(2, FIRST)
            emit_conv(2, SECOND)
            emit_blend_u(0, 0, BD)
            emit_blend_u(0, 1, BD)
            emit_conv(3, FIRST)
            emit_blend_u(1, 0, BD)
            emit_conv(3, SECOND)
            emit_blend_u(1, 1, BD)
            emit_conv(4, FIRST)
            emit_blend_u(2, 0, BD)
            emit_conv(4, SECOND)
            emit_blend_u(2, 1, BD)
            emit_conv(5, FIRST)
            emit_blend_u(3, 0, BD)
            emit_conv(5, SECOND)
            emit_blend_u(3, 1, BD)
            emit_blend_u(4, 0, BD)
            emit_conv(6, FIRST)
            emit_blend_u(4, 1, BD)
            emit_conv(6, SECOND)
            emit_blend_u(5, 0, BD)
            emit_blend_u(5, 1, BD)
            emit_conv(7, FIRST)
            emit_blend_u(6, 0, BD)
            emit_conv(7, SECOND)
            emit_blend_u(6, 1, BD)
            emit_blend_u(7, 0, BD)
            emit_blend_u(7, 1, BD)

    nc.compile()
    return nc


def pack_inputs(x, conv_w, conv_b, w1, b1, w2, b2):
    """Host-side layout packing (dtype casts, zero-padding, constant folds)."""
    import ml_dtypes

    bf16 = ml_dtypes.bfloat16
    x_bf = np.asarray(x, dtype=np.float32).reshape(B, CIN, HW, HW).astype(bf16)
    xpad = np.zeros((B, CIN, WP, WP), dtype=bf16)
    xpad[:, :, 1 : 1 + HW, 1 : 1 + HW] = x_bf

    # conv_w [K, COUT, CIN, 3, 3] -> [ci, t, tap, p] with p = c*4 + k,
    # co = 32 t + c
    w = np.asarray(conv_w, dtype=np.float32).transpose(2, 3, 4, 0, 1)  # ci kh kw k co
    w = w.reshape(CIN, KS, KS, K, 8, 32)  # ci kh kw k t c
    w = w.transpose(0, 4, 1, 2, 5, 3)  # ci t kh kw c k
    wconv = np.ascontiguousarray(w.reshape(CIN, 8 * 9 * 128)).astype(bf16)

    bc = np.asarray(conv_b, dtype=np.float32).reshape(K, 8, 32)  # k t c
    bconv = np.ascontiguousarray(bc.transpose(1, 2, 0).reshape(8, 128).T)  # [p, t]

    w1t = (np.ascontiguousarray(np.asarray(w1, dtype=np.float32).T) / float(HW2)).astype(bf16)
    b1c = np.ascontiguousarray(np.asarray(b1, dtype=np.float32).reshape(2, 128).T)
    w2T = np.asarray(w2, dtype=np.float32).T  # [256, 4]
    w2t = np.ascontiguousarray(np.concatenate([w2T[:128], w2T[128:]], axis=1)).astype(bf16)
    b2r = np.asarray(b2, dtype=np.float32).reshape(1, K).astype(bf16)

    p4 = np.zeros((K, 128), dtype=np.float32)
    m01 = np.zeros((128, 256), dtype=np.float32)
    for u in range(2):
        for c in range(16):
            for k in range(K):
                p4[k, 64 * u + 4 * c + k] = 1.0
                m01[64 * u + 4 * c + k, 128 * u + 16 * np.arange(8) + c] = 1.0

    common = dict(
        wconv=wconv, bconv=bconv, w1t=w1t, b1c=b1c, w2t=w2t, b2r=b2r,
        one18=np.ones((1, B), dtype=np.float32).astype(bf16),
        ident8=np.eye(B, dtype=np.float32).astype(bf16),
        p4=p4.astype(bf16), m01=m01.astype(bf16),
    )
    in_maps = [
        dict(
            common,
            xp0=np.ascontiguousarray(xpad[i].reshape(CIN, WP2)),
            xrest=np.ascontiguousarray(
                np.roll(x_bf, -i, axis=0)[1:].reshape(7 * CIN, HW2)
            ),
        )
        for i in range(NCORES)
    ]
    return in_maps


def run(inputs, trace=False):
    from concourse.bass_utils import run_bass_kernel_spmd

    nc = build_nc()
    in_maps = pack_inputs(**inputs)
    res = run_bass_kernel_spmd(
        nc, in_maps, core_ids=list(range(NCORES)), trace=trace
    )
    # core i's slab row q holds sample j = (i + q) % 8: un-rotate
    slabs = [np.roll(res.results[i]["out"], i, axis=0) for i in range(NCORES)]
    out = np.stack(slabs, axis=0).reshape(B, B, COUT, HW, HW)
    return out, res


def kernel(**inputs) -> np.ndarray:
    out, _ = run(inputs, trace=False)
    return out
